# revision 3
# baseline (speedup 1.0000x reference)
"""Single-head causal attention (B=8, T=2048, E=H=1024) on 8 TRN2 NeuronCores.

Strategy: data-parallel over batch (one batch element per core). Per core:
  warmup:   3 fp32 matmuls on a scratch tile keep the PE busy during the
            initial input DMA so the HAM clock-gate is already at 2.4 GHz
            when real work starts (cold PE runs at 1.2 GHz).
  phase A2: v = x@Wv.T [T,H] runs FIRST (x.T blocks stationary, Wv.T
            moving), resident in SBUF as bf16. Its first ~15us are
            DMA-paced, which is also true of any first phase; meanwhile
            the whole A1 working set (x.T resident + Wq/Wk stream)
            prefetches behind it on the same ring.
  phase A1: qT = (x@Wq.T).T and kT = (x@Wk.T).T  [H,T], contraction over
            E, hm outer / t4 inner (x.T resident, weights stream through
            2 rotating buffers). qT and kT are cast to bf16 and BOTH stay
            resident in SBUF — no DRAM spill.
  phase B:  causal flash attention over t-chunks of 256 in the S^T
            orientation: S^T[s,t] = sum_h kT[h,s]*qT[h,t] (bf16 operands,
            fp32 PSUM); softmax weights come out as wT[s_block, t] tiles
            feeding O[t,h] += wT.T @ v[s_block] directly. Row sums ride
            along as matmuls against a ones column. Diagonal masking uses
            one precomputed [128,128] bf16 triangle mask applied with a
            DVE multiply; the fully-masked t-half of the last s-block of
            each chunk is skipped (scores computed at N=128).

All input DMAs ride the sync HWDGE ring in exact consumption order (FIFO
per ring => strict priority), with the first tiles split small so the
first matmul only waits on ~0.5MB.

Projections run in float32r (full-rate ~12-bit-significand fp32); the
attention part runs in bf16 (~2e-3 end-to-end vs the fp32 reference,
correctness gate is 2e-2).
"""

import numpy as np

import concourse.bacc as bacc
import concourse.mybir as mybir
import concourse.tile as tile
from concourse.bass_utils import run_bass_kernel_spmd

B, T, E, H = 8, 2048, 1024, 1024
N_CORES = 8
SCALE = float(E) ** -0.5

DT = mybir.dt.float32r
BF = mybir.dt.bfloat16
F32 = mybir.dt.float32

TCB = 256            # phase-B t-chunk width
N_TCB = T // TCB     # 8
N_EB = E // 128      # 8  e-blocks
N_HB = H // 128      # 8  h-blocks
N_SB = T // 128      # 16 s-blocks


def build_program():
    nc = bacc.Bacc("TRN2", target_bir_lowering=False, debug=False,
                   num_devices=N_CORES)

    # host-prepped layouts: every DMA reads long contiguous runs
    xT_d = nc.declare_dram_parameter("xA", [4, 128, N_EB, 512], DT,
                                     isOutput=False)   # [t4][p][ek][t]
    xV_d = nc.declare_dram_parameter("xV", [8, 128, N_EB, 256], DT,
                                     isOutput=False)   # [t8][p][ek][t]
    wqT_d = nc.declare_dram_parameter("WqT", [N_HB, 128, N_EB, 128], DT,
                                      isOutput=False)  # [hm][p][ek][h]
    wkT_d = nc.declare_dram_parameter("WkT", [N_HB, 128, N_EB, 128], DT,
                                      isOutput=False)
    wvT_d = nc.declare_dram_parameter("WvT", [E, H], DT, isOutput=False)
    out_d = nc.declare_dram_parameter("out", [T, H], F32, isOutput=True)

    with tile.TileContext(nc) as tc:
        with (
            tc.tile_pool(name="misc", bufs=1) as pool_misc,
            tc.tile_pool(name="v", bufs=1) as pool_v,
            tc.tile_pool(name="kt", bufs=1) as pool_kt,
            tc.tile_pool(name="qt", bufs=1) as pool_qt,
        ):
            vt = [pool_v.tile([128, H], BF, tag=f"v{j}", name=f"v{j}")
                  for j in range(N_SB)]
            kt = [pool_kt.tile([128, T], BF, tag=f"kt{k}", name=f"kt{k}")
                  for k in range(N_HB)]
            qt = [pool_qt.tile([128, T], BF, tag=f"qt{k}", name=f"qt{k}")
                  for k in range(N_HB)]

            ones_f = pool_misc.tile([128, 1], F32, tag="ones_f", name="ones_f")
            ones_b = pool_misc.tile([128, 1], BF, tag="ones_b", name="ones_b")
            mask = pool_misc.tile([128, 128], BF, tag="mask", name="mask")
            dummy = pool_misc.tile([128, 512], F32, tag="dummy", name="dummy")
            nc.gpsimd.memset(ones_f[:], 1.0)
            nc.gpsimd.memset(dummy[:], 0.0)
            nc.gpsimd.memset(mask[:], 1.0)
            nc.vector.tensor_copy(ones_b[:], ones_f[:])
            # triangle mask: keep s <= t within a 128x128 block
            nc.gpsimd.affine_select(
                out=mask[:], in_=mask[:],
                compare_op=mybir.AluOpType.is_ge,
                fill=0.0, base=0, channel_multiplier=-1,
                pattern=[[1, 128]])

            # ---------------- phase A2: v (resident, bf16) -----------------
            with (
                tc.tile_pool(name="wv", bufs=1) as pool_wv,
                tc.tile_pool(name="xv0", bufs=1) as pool_xv0,
                tc.tile_pool(name="x2", bufs=2) as pool_x2,
                tc.tile_pool(name="pv", bufs=4, space="PSUM") as psum_v,
                tc.tile_pool(name="pd", bufs=1, space="PSUM") as psum_d,
            ):
                # PE warmup: 3 fp32 matmuls (~1.7us each cold) on scratch.
                # They depend only on the gpsimd memsets, so they run during
                # the initial input DMA and un-throttle the HAM clock gate.
                dummy_ps = psum_d.tile([1, 512], F32, tag="dummy_ps",
                                       name="dummy_ps")
                for i in range(3):
                    nc.tensor.matmul(dummy_ps[:], ones_f[:], dummy[:],
                                     start=True, stop=True)

                wvh = [[pool_wv.tile([128, 512], DT, tag=f"wv{k}_{hc}",
                                     name=f"wv{k}_{hc}") for hc in range(2)]
                       for k in range(N_EB)]
                xv0 = [pool_xv0.tile([128, 2, 256], DT, tag=f"xv0_{i}",
                                     name=f"xv0_{i}") for i in range(4)]
                xvt = {t8: pool_x2.tile([128, N_EB, 256], DT, tag="xv",
                                        name=f"xv_{t8}")
                       for t8 in range(1, 8)}

                def wv_dma(k, hc):
                    nc.sync.dma_start(
                        wvh[k][hc][:],
                        wvT_d[k * 128:(k + 1) * 128,
                              hc * 512:(hc + 1) * 512])

                # sync-ring DMA triggers in consumption order: first t8-leg
                # pieces interleaved, then the rest
                wv_dma(0, 0)
                nc.sync.dma_start(xv0[0][:], xV_d[0, :, 0:2, :])
                wv_dma(1, 0)
                nc.sync.dma_start(xv0[1][:], xV_d[0, :, 2:4, :])
                wv_dma(2, 0)
                wv_dma(3, 0)
                nc.sync.dma_start(xv0[2][:], xV_d[0, :, 4:6, :])
                for k in range(4, N_EB):
                    wv_dma(k, 0)
                nc.sync.dma_start(xv0[3][:], xV_d[0, :, 6:8, :])
                for k in range(N_EB):
                    wv_dma(k, 1)
                nc.sync.dma_start(xvt[1][:], xV_d[1, :, :, :])
                nc.sync.dma_start(xvt[2][:], xV_d[2, :, :, :])

                def xv_slice(t8, ek, sl):
                    if t8 == 0:
                        return xv0[ek // 2][:, ek % 2, sl]
                    return xvt[t8][:, ek, sl]

                with nc.named_scope("proj_v"):
                    for t8 in range(T // 256):
                        if t8 >= 3:
                            # rest of the stream, trigger order == use order
                            nc.sync.dma_start(xvt[t8][:], xV_d[t8, :, :, :])
                        for ss in range(2):
                            j = t8 * 2 + ss
                            sl = slice(ss * 128, (ss + 1) * 128)
                            for hc in range(2):
                                pv = psum_v.tile([128, 512], F32, tag="pv",
                                                 name=f"pv_{t8}_{ss}_{hc}")
                                for ek in range(N_EB):
                                    nc.tensor.matmul(
                                        pv[:], xv_slice(t8, ek, sl),
                                        wvh[ek][hc][:],
                                        start=(ek == 0), stop=(ek == N_EB - 1))
                                if hc == 0:
                                    nc.vector.tensor_copy(
                                        vt[j][:, hc * 512:(hc + 1) * 512], pv[:])
                                else:
                                    nc.scalar.copy(
                                        vt[j][:, hc * 512:(hc + 1) * 512], pv[:])

            # ---------------- phase A1: qT + kT (both resident) ------------
            with (
                tc.tile_pool(name="xf", bufs=1) as pool_xf,
                tc.tile_pool(name="wqk", bufs=2) as pool_wqk,
                tc.tile_pool(name="pa", bufs=3, space="PSUM") as psum_a,
            ):
                xft = [pool_xf.tile([128, N_EB, 512], DT, tag=f"xf{t4}",
                                    name=f"xf{t4}") for t4 in range(4)]
                wqt = {}
                wkt = {}
                for hm in range(N_HB):
                    wqt[hm] = pool_wqk.tile([128, N_EB, 128], DT, tag="wqb",
                                            name=f"wqb{hm}")
                    wkt[hm] = pool_wqk.tile([128, N_EB, 128], DT, tag="wkb",
                                            name=f"wkb{hm}")

                # A1 loads ride the ring behind A2's; they land during A2's
                # compute. W tiles hm>=2 are gated by buffer rotation, so
                # their triggers are emitted inside the loop.
                nc.sync.dma_start(wqt[0][:], wqT_d[0, :, :, :])
                nc.sync.dma_start(xft[0][:], xT_d[0, :, :, :])
                nc.sync.dma_start(wkt[0][:], wkT_d[0, :, :, :])
                nc.sync.dma_start(wqt[1][:], wqT_d[1, :, :, :])
                nc.sync.dma_start(wkt[1][:], wkT_d[1, :, :, :])
                for t4 in range(1, 4):
                    nc.sync.dma_start(xft[t4][:], xT_d[t4, :, :, :])

                with nc.named_scope("proj_qk"):
                    for hm in range(N_HB):
                        if hm + 2 < N_HB:
                            nc.sync.dma_start(wqt[hm + 2][:],
                                              wqT_d[hm + 2, :, :, :])
                            nc.sync.dma_start(wkt[hm + 2][:],
                                              wkT_d[hm + 2, :, :, :])
                        for t4 in range(4):
                            pq = psum_a.tile([128, 512], F32, tag="pq",
                                             name=f"pq_{hm}_{t4}")
                            pk = psum_a.tile([128, 512], F32, tag="pk",
                                             name=f"pk_{hm}_{t4}")
                            for ek in range(N_EB):
                                nc.tensor.matmul(
                                    pq[:], wqt[hm][:, ek, :],
                                    xft[t4][:, ek, :],
                                    start=(ek == 0), stop=(ek == N_EB - 1))
                            for ek in range(N_EB):
                                nc.tensor.matmul(
                                    pk[:], wkt[hm][:, ek, :],
                                    xft[t4][:, ek, :],
                                    start=(ek == 0), stop=(ek == N_EB - 1))
                            nc.scalar.copy(
                                qt[hm][:, t4 * 512:(t4 + 1) * 512], pq[:])
                            nc.vector.tensor_copy(
                                kt[hm][:, t4 * 512:(t4 + 1) * 512], pk[:])

            # ---------------- phase B: causal attention --------------------
            with (
                tc.tile_pool(name="wt", bufs=3) as pool_wt,
                tc.tile_pool(name="ob", bufs=4) as pool_ob,
                tc.tile_pool(name="sm", bufs=4) as pool_sm,
                tc.tile_pool(name="pb", bufs=1, space="PSUM") as psum_b,
            ):
                with nc.named_scope("attn"):
                    for c in range(N_TCB):
                        n_j = 2 * c + 2
                        o_ps = [psum_b.tile([128, 512], F32, tag=f"O{i}",
                                            name=f"O_{c}_{i}")
                                for i in range(4)]
                        rs_ps = psum_b.tile([1, TCB], F32, tag="rs",
                                            name=f"rs_{c}")

                        def scores(j, c=c, n_j=n_j):
                            # last s-block of the chunk: t-half 0 is fully
                            # masked -> only compute the 128 t-half-1 cols
                            half = (j == n_j - 1)
                            off = 128 if half else 0
                            s_ps = psum_b.tile([128, TCB], F32,
                                               tag=f"S{j % 2}",
                                               name=f"S_{c}_{j}")
                            for hk in range(N_HB):
                                nc.tensor.matmul(
                                    s_ps[:, off:TCB],
                                    kt[hk][:, j * 128:(j + 1) * 128],
                                    qt[hk][:, c * TCB + off:(c + 1) * TCB],
                                    start=(hk == 0), stop=(hk == N_HB - 1))
                            wt = pool_wt.tile([128, TCB], BF, tag="wt",
                                              name=f"wt_{c}_{j}")
                            nc.scalar.activation(
                                wt[:, off:TCB], s_ps[:, off:TCB],
                                mybir.ActivationFunctionType.Exp,
                                scale=SCALE)
                            if j == 2 * c:
                                # diagonal block: t-half 0 is triangular
                                nc.vector.tensor_mul(
                                    wt[:, 0:128], wt[:, 0:128], mask[:])
                            elif half:
                                # block j=2c+1: t-half 1 is triangular
                                nc.vector.tensor_mul(
                                    wt[:, 128:TCB], wt[:, 128:TCB], mask[:])
                            return wt

                        def o_accum(j, wt, c=c, n_j=n_j, o_ps=o_ps,
                                    rs_ps=rs_ps):
                            first, last = (j == 0), (j == n_j - 1)
                            off = 128 if last else 0
                            nc.tensor.matmul(
                                rs_ps[0:1, off:TCB], ones_b[:],
                                wt[:, off:TCB],
                                start=first, stop=last,
                                skip_group_check=True)
                            for ts in range(2):
                                if ts == 0 and last:
                                    # fully-masked: all-zero contribution
                                    continue
                                wslice = wt[:, ts * 128:(ts + 1) * 128]
                                last_ts = (j == n_j - 2) if ts == 0 \
                                    else last
                                for hc in range(2):
                                    nc.tensor.matmul(
                                        o_ps[ts * 2 + hc][:], wslice,
                                        vt[j][:, hc * 512:(hc + 1) * 512],
                                        start=first, stop=last_ts)

                        # software pipeline: scores(j+1) issued ahead of
                        # O(j) so the PE never waits on the exp chain
                        wt_cur = scores(0)
                        for j in range(n_j):
                            wt_next = scores(j + 1) if j + 1 < n_j else None
                            o_accum(j, wt_cur)
                            wt_cur = wt_next
                        rs_sb = pool_sm.tile([1, TCB], F32, tag="rs_sb",
                                             name=f"rs_sb_{c}")
                        nc.vector.tensor_copy(rs_sb[:], rs_ps[:])
                        for ts in range(2):
                            # transpose [1,128] -> [128,1] via K=1 matmul
                            rs_col = psum_b.tile([128, 1], F32,
                                                 tag="rs_col",
                                                 name=f"rs_col_{c}_{ts}")
                            nc.tensor.matmul(
                                rs_col[:],
                                rs_sb[0:1, ts * 128:(ts + 1) * 128],
                                ones_f[0:1, 0:1],
                                start=True, stop=True)
                            rec = pool_sm.tile([128, 1], F32, tag="rec",
                                               name=f"rec_{c}_{ts}")
                            nc.vector.reciprocal(rec[:], rs_col[:])
                            for hc in range(2):
                                ob = pool_ob.tile([128, 512], F32, tag="ob",
                                                  name=f"ob_{c}_{ts}_{hc}")
                                if hc == 0:
                                    nc.vector.tensor_scalar_mul(
                                        ob[:], o_ps[ts * 2 + hc][:], rec[:])
                                else:
                                    nc.scalar.activation(
                                        ob[:], o_ps[ts * 2 + hc][:],
                                        mybir.ActivationFunctionType.Copy,
                                        scale=rec[:])
                                out_ap = out_d[c * TCB + ts * 128:
                                               c * TCB + (ts + 1) * 128,
                                               hc * 512:(hc + 1) * 512]
                                if c == N_TCB - 1:
                                    nc.sync.dma_start(out_ap, ob[:])
                                else:
                                    nc.gpsimd.dma_start(out_ap, ob[:])

    nc.compile()
    return nc


_NC_CACHE = None


def _get_program():
    global _NC_CACHE
    if _NC_CACHE is None:
        _NC_CACHE = build_program()
    return _NC_CACHE


def make_in_maps(x, Wk, Wq, Wv):
    x = np.asarray(x, np.float32)
    xT = np.transpose(x, (0, 2, 1))                        # [B, E, T]
    # A1 layout [t4][p][ek][512]: xT[e, t] with e = ek*128 + p
    xA = np.ascontiguousarray(
        xT.reshape(B, N_EB, 128, 4, 512).transpose(0, 3, 2, 1, 4))
    # A2 layout [t8][p][ek][256]
    xV = np.ascontiguousarray(
        xT.reshape(B, N_EB, 128, 8, 256).transpose(0, 3, 2, 1, 4))

    def prep_w(W):   # [H,E] -> W.T [E,H] -> [hm][p][ek][128]
        WT = np.asarray(W, np.float32).T
        return np.ascontiguousarray(
            WT.reshape(N_EB, 128, N_HB, 128).transpose(2, 1, 0, 3))

    WqT = prep_w(Wq)
    WkT = prep_w(Wk)
    WvT = np.ascontiguousarray(np.asarray(Wv, np.float32).T)  # [E, H]
    return [{"xA": xA[b], "xV": xV[b], "WqT": WqT, "WkT": WkT, "WvT": WvT}
            for b in range(B)]


def kernel(x, Wk, Wq, Wv, _trace=False, _tmpdir=None):
    nc = _get_program()
    in_maps = make_in_maps(x, Wk, Wq, Wv)
    res = run_bass_kernel_spmd(nc, in_maps, list(range(N_CORES)),
                               trace=_trace, tmpdir=_tmpdir)
    out = np.stack([res.results[b]["out"] for b in range(B)])
    if _trace:
        kernel.last_result = res
    return out
